# revision 7
# baseline (speedup 1.0000x reference)
"""Trainium2 Bass kernel for ExemplarGNN2AdjModel (gnn_message_passing).

Math:
  h  = relu(relu(x@W1+b1)@W2+b2)                      # [512,128] node encoder
  scores[i,j] = Wp2 . relu(Wp1a.h_i + Wp1b.h_j + Wp1c.|h_i-h_j| + bp1) + bp2

Device algorithm (per core, SPMD over 8 cores; core c handles 64 rows of i):
  - Each core receives x pre-rolled by c*64 rows and pre-transposed (xT), so the
    identical program computes rows [c*64, c*64+64) in its local (rolled) node
    order; the host un-rolls the output columns afterwards.
  - |h_i-h_j| = h_i + h_j - 2*min(h_i,h_j): the h_i term is folded into the
    per-i bias matrix (wp1a += w3), the h_j term into the B matmul
    (w2p += w3), and the per-pair part is -2*w3^T min(h_i, h_j).
  - w3^{-1} fold: M = (w2p w3^{-1})^T h satisfies w3^T M = w2p^T h, so
      P_r = w3^T (min(h, h_r) + M)
    needs ONE matmul per row instead of two.  The fused DVE/GPSIMD op
    scalar_tensor_tensor computes (h min h_r) add M in one pass.  M is ~10x
    larger than h, so the sum is stored in fp16 (10 mantissa bits); the whole
    kernel runs fp16 (same PE/DVE rate as bf16, strictly better precision at
    these magnitudes).  Verified numerically: rel err ~2.2e-3 (same as the
    bf16 two-matmul baseline).
  - Engine balance per 4-row group (the steady-state wall is ~1.8us on PE,
    DVE and ACT simultaneously):
      rows r0,r1: classic two-matmul accumulate (w2p start + w3 stop), plain
                  DVE min for d.
      row  r2:    fused stt on DVE + single w3 matmul.
      row  r3:    fused stt on GPSIMD (no PSUM access, SBUF-only op) + single
                  w3 matmul.
      relus: r0,r1,r2 on ACT (bias=A2 col), r3 on DVE (tensor_scalar add,max).
  - Encoder runs on-device in fp16 (all 512 nodes, replicated per core),
    fp32 PSUM accumulation, fp32 biases.  A2 = wp1a^T h + bp1 and the M
    matmul + PSUM->SBUF fp16 copy happen once in the ramp.
  - The 64 rows are processed in 16 groups of 4, one row from each of the four
    16-row output blocks (i, i+16, i+32, i+48).  d/dp tiles for group g+1 are
    emitted at the start of group g (1 group of lookahead).
  - out[16b+i,:] += embW_r^T hid_r: 4 col-tiled PE matmuls to PSUM partitions
    0-15/32-47/64-79/96-111 -- disjoint col_grp strips run concurrently
    (~1 slot for all 4).  embW_r = embbuf[:, 15-i : 31-i]: a sliding window
    over a 31-column zero buffer with Wp2 at column 15 puts Wp2 exactly in
    stationary column i.  Out matmuls of group g are issued in group g+2 so
    the in-order PE never waits on a relu.
  - Output: two accumulation phases of 8 groups in TWO separate PSUM banks
    (outp0/outp1), so phase 1 never waits on the phase-0 flush.  bp2 is added
    on the host after the gather.  Phase-0 flush: PSUM->SBUF copy in two
    column-halves on DVE during groups 10-11 (GPSIMD absorbs the DVE stt in
    those groups), then 4 strip DMAs on sync/gpsimd.  Phase-1 flush at the
    tail: copy halves on DVE+ACT in parallel (both idle), 4 strip DMAs with
    descriptor-gen spread across the sync/scalar/vector/tensor queues (all
    idle at the tail; gpsimd's SWDGE drain costs ~2us at kernel end).
  - Startup: xtp is DMA'd in 5 k-chunks with doorbells spread across the sync/
    gpsimd queues (doorbells cost ~600ns each and serialize per queue);
    encoder matmuls start as chunks land, with narrow [0:128] first-pieces of
    relu1/h2/hbf so the hT/min chain launches early; small dummy matmuls
    bridge every DMA/relu gap so the PE HAM clock-gate stays at 2.4 GHz.
"""

import numpy as np

B = 512
IN_DIM = 595
HID = 128
NCORES = 8
RPC = B // NCORES  # rows per core = 64
NBLK = 4           # output col-tile blocks
BLK = RPC // NBLK  # 16 rows per block
DEFER_G = 2        # groups between producing hid and its out matmul
N_WARM_MM = 8      # dummy matmuls: sustained PE activity trips the HAM
                   # clock-gate to 2.4 GHz before the encoder matmuls run
WARM_N = 256       # free dim of warm matmuls

# in_dim k-tiles for the first encoder matmul (contraction over 595)
KT = [(0, 128), (128, 256), (256, 384), (384, 512), (512, 595)]

_PROGRAM_CACHE = {}


def _build_program():
    import concourse.mybir as mybir
    import concourse.tile as tile
    from concourse import bacc

    f32 = mybir.dt.float32
    f16 = mybir.dt.float16
    Act = mybir.ActivationFunctionType
    Alu = mybir.AluOpType

    nc = bacc.Bacc("TRN2", target_bir_lowering=False)

    NKT = len(KT)
    xt_d = nc.dram_tensor("xtp", [HID, NKT * B], f16, kind="ExternalInput")
    w1_d = nc.dram_tensor("w1p", [HID, NKT * HID], f16, kind="ExternalInput")
    wpack_d = nc.dram_tensor("wpack", [HID, 5 * HID], f16, kind="ExternalInput")
    bias_d = nc.dram_tensor("biases", [HID, 4], f32, kind="ExternalInput")
    out_d = nc.dram_tensor("out", [RPC, B], f32, kind="ExternalOutput")

    with tile.TileContext(nc) as tc:
        with (
            tc.tile_pool(name="consts", bufs=1) as consts,
            tc.tile_pool(name="setup", bufs=1) as setup,
            tc.tile_pool(name="dwork", bufs=10) as dwork,
            tc.tile_pool(name="hwork", bufs=13) as hwork,
            tc.tile_pool(name="penc", bufs=2, space="PSUM") as penc,
            tc.tile_pool(name="ppair", bufs=6, space="PSUM") as ppair,
        ):
            # ---- input loads first: doorbells cost ~600ns each and serialize
            # per queue, so spread the xtp chunks across three idle queues.
            xt_all = consts.tile([HID, NKT * B], f16)
            w1_all = consts.tile([HID, NKT * HID], f16)
            biases = consts.tile([HID, 4], f32)
            wpack = consts.tile([HID, 5 * HID], f16)
            # earliest-needed first; k-chunks split across sync/gpsimd queues
            nc.scalar.dma_start(out=w1_all, in_=w1_d[:, :])
            qeng = [nc.sync, nc.gpsimd, nc.sync, nc.gpsimd, nc.sync]
            for k in range(NKT):
                qeng[k].dma_start(
                    out=xt_all[:, k * B : (k + 1) * B],
                    in_=xt_d[:, k * B : (k + 1) * B],
                )
            nc.scalar.dma_start(out=biases, in_=bias_d[:, :])
            nc.scalar.dma_start(out=wpack, in_=wpack_d[:, :])

            # ---- PE warm-up over the DMA window (HAM ramps to 2.4 GHz)
            scratch = setup.tile([HID, B], f16)
            nc.vector.memset(scratch, 0.0)
            scratch1 = setup.tile([HID, 1], f32)
            nc.scalar.activation(scratch1, scratch[:, 0:1], Act.Relu)

            def warm_mm(n, w=WARM_N):
                # dummy matmuls keep the PE busy (HAM clock-gate stays at
                # 2.4 GHz) across DMA-wait and relu-wait gaps; they use ppair
                # banks so they never touch the encoder/out accumulator bank
                for _ in range(n):
                    wp = ppair.tile([HID, B], f32, name="pp")
                    nc.tensor.matmul(
                        wp[:, 0:w], lhsT=scratch[:, 0:HID], rhs=scratch[:, 0:w],
                        start=True, stop=True, skip_group_check=True,
                    )

            warm_mm(N_WARM_MM)

            # sliding-window Wp2 buffer: zeros with Wp2 at column BLK-1; the
            # out matmul for block-row i uses embbuf[:, BLK-1-i+c] == Wp2 iff
            # c == i.
            embbuf = consts.tile([HID, 2 * BLK - 1], f16)
            nc.vector.memset(embbuf, 0.0)

            xt_sb = [xt_all[:, k * B : (k + 1) * B] for k in range(NKT)]
            w1_sb = [w1_all[:, k * HID : (k + 1) * HID] for k in range(NKT)]
            w2_sb = wpack[:, 0 * HID : 1 * HID]
            wp1a_sb = wpack[:, 1 * HID : 2 * HID]
            w2p_sb = wpack[:, 2 * HID : 3 * HID]
            w3_sb = wpack[:, 3 * HID : 4 * HID]
            g_sb = wpack[:, 4 * HID : 5 * HID]
            b1_sb = biases[:, 0:1]
            b2_sb = biases[:, 1:2]
            bp1_sb = biases[:, 2:3]

            # ---- encoder: h1 = relu(W1^T xT + b1), hT = relu(W2^T h1 + b2) ----
            h1p = penc.tile([HID, B], f32, name="encp", tag="encp")
            for k in range(len(KT)):
                nc.tensor.matmul(
                    h1p, lhsT=w1_sb[k], rhs=xt_sb[k],
                    start=(k == 0), stop=(k == len(KT) - 1),
                )
                if k > 0:
                    warm_mm(1)  # bridge the DMA-gated gaps between k-chunks
            # encoder relus split: a narrow ACT first-piece [0:128] lets the
            # h2 -> hbf -> hT -> min chain start early; DVE takes the rest
            HQ = B // 4
            HB2 = B // 2
            h1bf = setup.tile([HID, B], f16)
            nc.scalar.activation(h1bf[:, 0:HQ], h1p[:, 0:HQ], Act.Relu, bias=b1_sb)
            nc.vector.tensor_scalar(
                h1bf[:, HQ:B], h1p[:, HQ:B], b1_sb, 0.0, Alu.add, Alu.max
            )

            # h2 in two matmuls to different PSUM banks: a narrow first piece
            # [0:128] feeding ACT, the rest feeding DVE, so the hT/min chain
            # starts as early as possible
            h2p = penc.tile([HID, HQ], f32, name="encp2", tag="encp")
            nc.tensor.matmul(h2p, lhsT=w2_sb, rhs=h1bf[:, 0:HQ], start=True, stop=True)
            h2pb = ppair.tile([HID, B], f32, name="pp")
            nc.tensor.matmul(
                h2pb[:, 0 : B - HQ], lhsT=w2_sb, rhs=h1bf[:, HQ:B],
                start=True, stop=True, skip_group_check=True,
            )
            warm_mm(2)  # bridge PE over relu2 + hT
            hbf = setup.tile([HID, B], f16)
            nc.scalar.activation(hbf[:, 0:HQ], h2p, Act.Relu, bias=b2_sb)
            nc.vector.tensor_scalar(
                hbf[:, HQ:B], h2pb[:, 0 : B - HQ], b2_sb, 0.0, Alu.add, Alu.max
            )
            # hT fp32 is the per-row scalar operand of the min (tensor_scalar
            # scalars must be fp32); only the core's 64 local-row columns are
            # ever read, and deriving it from hbf avoids a second serialized
            # read of the h2p PSUM bank
            hT = setup.tile([HID, RPC], f32)
            nc.vector.tensor_copy(hT, hbf[:, 0:RPC])
            # Wp2 rides in as f32 column 3 of biases; cast into the sliding
            # window buffer.  Emitted HERE (needed only by the first out
            # matmuls): emitting it earlier head-of-line-blocks the in-order
            # DVE queue on the biases DMA and delays the encoder relus ~1us.
            nc.vector.tensor_copy(embbuf[:, BLK - 1 : BLK], biases[:, 3:4])

            # ---- A2 = wp1a^T h + bp1  (per-i relu bias columns) ----
            a2p = penc.tile([HID, B], f32, name="encp3", tag="encp")
            nc.tensor.matmul(a2p, lhsT=wp1a_sb, rhs=hbf, start=True, stop=True)
            warm_mm(1)
            # ---- M = (w2p w3^{-1})^T h, then PSUM -> SBUF fp16 ----
            mp = penc.tile([HID, B], f32, name="encp4", tag="encp")
            nc.tensor.matmul(mp, lhsT=g_sb, rhs=hbf, start=True, stop=True)
            warm_mm(1)  # bridge PE over the first mins
            a2 = setup.tile([HID, B], f32)
            nc.scalar.activation(a2, a2p, Act.Identity, bias=bp1_sb)
            msb = setup.tile([HID, B], f16)
            nc.vector.tensor_copy(msb, mp)

            # per-row d/dp production, one group of lookahead.
            # mode: r0,r1 -> plain min (PE does w2p accumulate); r2 -> fused
            # (min h,h_r)+M stt on DVE; r3 -> DVE min + GPSIMD tensor_tensor
            # add (the Pool engine rejects TensorScalarPtr instructions, so
            # the fused stt cannot run there).
            dtiles = {}

            def emit_d(g):
                if not (0 <= g < BLK):
                    return
                r0, r1, r2, r3 = rows_of(g)
                # r3's min goes first on the DVE queue: the GPSIMD add that
                # consumes it is slow (~1.1us) and must start early to have
                # dp_r3 ready for next group's matmul
                d3 = dwork.tile([HID, B], f16, name="dtile")
                nc.vector.tensor_scalar(d3, hbf, hT[:, r3 : r3 + 1], None, Alu.min)
                dp3 = dwork.tile([HID, B], f16, name="dtile")
                nc.gpsimd.tensor_tensor(dp3, d3, msb, Alu.add)
                dtiles[r3] = dp3
                for r in (r0, r1):
                    d = dwork.tile([HID, B], f16, name="dtile")
                    nc.vector.tensor_scalar(
                        d, hbf, hT[:, r : r + 1], None, Alu.min
                    )
                    dtiles[r] = d
                d2 = dwork.tile([HID, B], f16, name="dtile")
                nc.vector.scalar_tensor_tensor(
                    d2, hbf, hT[:, r2 : r2 + 1], msb, Alu.min, Alu.add
                )
                dtiles[r2] = d2

            # out accumulation: a single 16-group phase into one PSUM bank;
            # group g writes partition 32b + g of block b's col_grp strip.
            # The 4 blocks hit disjoint col_grp strips of the PE array and
            # their out matmuls run concurrently.
            PH = BLK  # 16 groups, one phase
            outp = penc.tile([HID, B], f32, name="outp", tag="encp")

            def rows_of(g):
                return [g + BLK * b for b in range(NBLK)] if 0 <= g < BLK else []

            pending = {}

            def emit_outs(g):
                for b in range(NBLK):
                    r = g + BLK * b
                    hid_r = pending.pop(r)
                    nc.tensor.matmul(
                        outp[32 * b : 32 * b + PH, :],
                        lhsT=embbuf[:, BLK - 1 - g : BLK - 1 - g + PH],
                        rhs=hid_r,
                        start=(g == 0), stop=(g == PH - 1),
                        skip_group_check=True,
                        tile_position=(0, 32 * b),
                    )

            # prime the d pipeline
            emit_d(0)

            # ---- pairwise main loop: 16 groups of 4 rows ----
            for g in range(BLK):
                rows = rows_of(g)
                emit_d(g + 1)
                # PE slots: w2p starts for r0,r1; single-mm rows r2,r3; the
                # deferred outs sit between so the same-bank accumulate pairs
                # (r0,r1) get 3-4 slots of spacing before their stop halves
                pps = []
                for r in rows[:2]:
                    pp = ppair.tile([HID, B], f32, name="pp")
                    nc.tensor.matmul(
                        pp, lhsT=w2p_sb, rhs=hbf,
                        start=True, stop=False, skip_group_check=True,
                    )
                    pps.append(pp)
                pp2 = ppair.tile([HID, B], f32, name="pp")
                nc.tensor.matmul(
                    pp2, lhsT=w3_sb, rhs=dtiles.pop(rows[2]),
                    start=True, stop=True, skip_group_check=True,
                )
                if g - DEFER_G >= 0:
                    emit_outs(g - DEFER_G)
                pp3 = ppair.tile([HID, B], f32, name="pp")
                nc.tensor.matmul(
                    pp3, lhsT=w3_sb, rhs=dtiles.pop(rows[3]),
                    start=True, stop=True, skip_group_check=True,
                )
                for r, pp in zip(rows[:2], pps):
                    nc.tensor.matmul(
                        pp, lhsT=w3_sb, rhs=dtiles.pop(r),
                        start=False, stop=True, skip_group_check=True,
                    )
                pps = pps + [pp2, pp3]
                # relus: r0,r1,r2 on ACT, r3 on DVE.  In the last group the
                # DVE has no d tiles left to produce, so split 2/2 to shorten
                # the chain gating the final out matmuls.
                act_blocks = 2 if g == BLK - 1 else 3
                for bi, (r, pp) in enumerate(zip(rows, pps)):
                    hid = hwork.tile([HID, B], f16, name="hid")
                    if bi < act_blocks:
                        nc.scalar.activation(
                            hid, pp, Act.Relu, bias=a2[:, r : r + 1]
                        )
                    else:
                        nc.vector.tensor_scalar(
                            hid, pp, a2[:, r : r + 1], 0.0, Alu.add, Alu.max
                        )
                    pending[r] = hid
            for g in range(BLK - DEFER_G, BLK):
                emit_outs(g)
            # tail flush: copy column-halves in parallel on DVE + ACT (both
            # idle at the tail; ACT has no tensor_copy so it uses Identity);
            # bp2 is added on the host after the gather.  Descriptor gen for
            # the 4 strip DMAs is split across the sync/scalar queues (only
            # sync/scalar/gpsimd can issue DMAs, and gpsimd is kept strictly
            # off the tail: measured +9us regression from its SWDGE drain
            # there).
            o = setup.tile([HID, B], f32, name="outs")
            nc.vector.tensor_copy(o[:, 0:HB2], outp[:, 0:HB2])
            nc.scalar.activation(o[:, HB2:B], outp[:, HB2:B], Act.Identity)
            fq = [nc.sync, nc.scalar, nc.sync, nc.scalar]
            for b in range(NBLK):
                fq[b].dma_start(
                    out=out_d[BLK * b : BLK * (b + 1), :],
                    in_=o[32 * b : 32 * b + BLK, :],
                )

    nc.finalize()
    return nc


def _get_program():
    if "nc" not in _PROGRAM_CACHE:
        _PROGRAM_CACHE["nc"] = _build_program()
    return _PROGRAM_CACHE["nc"]


def _make_in_maps(x, W1, b1, W2, b2, Wp1, bp1, Wp2, bp2):
    f16 = np.float16
    f32 = np.float32
    f64 = np.float64
    x = np.asarray(x, dtype=f32)
    W1 = np.asarray(W1, dtype=f32)
    W2 = np.asarray(W2, dtype=f32)
    Wp1 = np.asarray(Wp1, dtype=f64)
    Wp2 = np.asarray(Wp2, dtype=f32).reshape(HID, 1)
    b1c = np.ascontiguousarray(np.asarray(b1, dtype=f32).reshape(HID, 1))
    b2c = np.ascontiguousarray(np.asarray(b2, dtype=f32).reshape(HID, 1))
    bp1c = np.ascontiguousarray(np.asarray(bp1, dtype=f32).reshape(HID, 1))

    # |h_i - h_j| = h_i + h_j - 2*min(h_i, h_j) folds (see module docstring)
    w3f = Wp1[2 * HID : 3 * HID, :]
    wp1a = Wp1[0:HID, :] + w3f
    w2p = Wp1[HID : 2 * HID, :] + w3f
    w3 = -2.0 * w3f
    # w3^{-1} fold: M = G^T h with G = w2p w3^{-1} satisfies w3^T M = w2p^T h
    G = w2p @ np.linalg.inv(w3)

    NKT = len(KT)
    KPAD = NKT * HID  # 640: in_dim padded so every k-tile is 128 partitions

    # packed weights [w2 | wp1a | w2p | w3 | G], biases [b1 | b2 | bp1 | wp2]
    wpack = np.concatenate(
        [W2.astype(f64), wp1a, w2p, w3, G], axis=1
    ).astype(f16)
    biases = np.zeros((HID, 4), dtype=f32)
    biases[:, 0:1] = b1c
    biases[:, 1:2] = b2c
    biases[:, 2:3] = bp1c
    biases[:, 3:4] = Wp2

    # w1 padded to [640, 128], viewed as [128, 5*128]
    w1_pad = np.zeros((KPAD, HID), dtype=f32)
    w1_pad[:IN_DIM] = np.asarray(W1, dtype=f32)
    w1p = np.ascontiguousarray(
        w1_pad.reshape(NKT, HID, HID).transpose(1, 0, 2).reshape(HID, NKT * HID)
    ).astype(f16)

    shared = dict(w1p=w1p, wpack=wpack, biases=biases)
    in_maps = []
    for c in range(NCORES):
        xr = np.roll(x, -c * RPC, axis=0)
        xt_pad = np.zeros((KPAD, B), dtype=f32)
        xt_pad[:IN_DIM] = xr.T
        xtp = np.ascontiguousarray(
            xt_pad.reshape(NKT, HID, B).transpose(1, 0, 2).reshape(HID, NKT * B)
        ).astype(f16)
        m = dict(shared)
        m["xtp"] = xtp
        in_maps.append(m)
    return in_maps


def _run(in_maps, trace=False):
    from concourse.bass_utils import run_bass_kernel_spmd

    nc = _get_program()
    return run_bass_kernel_spmd(
        nc, in_maps, core_ids=list(range(NCORES)), trace=trace
    )


def kernel(x, W1, b1, W2, b2, Wp1, bp1, Wp2, bp2):
    in_maps = _make_in_maps(x, W1, b1, W2, b2, Wp1, bp1, Wp2, bp2)
    res = _run(in_maps, trace=False)
    bp2v = np.float32(np.asarray(bp2, dtype=np.float32).reshape(-1)[0])
    out = np.empty((B, B), dtype=np.float32)
    for c in range(NCORES):
        blk = np.asarray(res.results[c]["out"], dtype=np.float32)
        # device block row r*BLK.. maps rows (g + BLK*b); device row order is
        # [g + 16b] = natural order, so rows are already 0..63
        out[c * RPC : (c + 1) * RPC, :] = np.roll(blk, c * RPC, axis=1) + bp2v
    return out
